# revision 4
# baseline (speedup 1.0000x reference)
"""PointUpsampleAttn (3-NN gather attention) Trainium2 kernel.

Full-input contract: kernel(q, k, v) -> [B, C, N] float32.
  q [4, 16384, 3], k [4, 4096, 3], v [4, 4096, 256]

Sharding: B*N = 65536 queries split across 8 cores (8192 each); core c
handles batch c//2, query half c%2. k/v replicated per-batch (each core
only needs its own batch's k/v). No cross-core reduction.

Per-core kernel, per 128-query tile:
  1. PE matmul (K=11, fp16 hi/lo split of q / 2k / -|k|^2) -> m = 2qk-kk
     in PSUM [128, 4096], fp32-class accuracy at full bf16 PE rate.
  2. ACT copies PSUM -> SBUF.
  3. DVE max8 + max_index -> top-3 m values + s-indices.
  4. weights w = normalize(1/(qq+eps-m_top3)).
  5. 3x indirect DMA gather of v rows; weighted sum; PE transpose to
     [C, n] layout; DMA to output.
"""

import numpy as np

B, N, S, C = 4, 16384, 4096, 256
NCORES = 8
NSH = (B * N) // NCORES   # 8192 queries per core
PT = 128                  # queries per tile (partition dim)
NT = NSH // PT            # 64 tiles
KROWS = 21                # contraction rows of the split matmul

_CACHE = {}


def _build_bass():
    import concourse.bacc as bacc
    import concourse.mybir as mybir
    import concourse.tile as tile
    from concourse import bass
    from concourse.masks import make_identity

    f32 = mybir.dt.float32
    f16 = mybir.dt.float16
    u32 = mybir.dt.uint32

    nc = bacc.Bacc("TRN2", target_bir_lowering=False, debug=False)

    a_d = nc.dram_tensor("a", [KROWS, NSH], f16, kind="ExternalInput").ap()
    k_d = nc.dram_tensor("kaug", [KROWS, S], f16, kind="ExternalInput").ap()
    qq_d = nc.dram_tensor("qq", [PT, NT], f32, kind="ExternalInput").ap()
    v_d = nc.dram_tensor("v", [S, C], f32, kind="ExternalInput").ap()
    out_d = nc.dram_tensor("out", [C, NSH], f32, kind="ExternalOutput").ap()

    with tile.TileContext(nc) as tc:
        with (
            tc.tile_pool(name="const", bufs=1) as cpool,
            tc.tile_pool(name="m", bufs=2) as mpool,
            tc.tile_pool(name="sel", bufs=3) as spool,
            tc.tile_pool(name="g", bufs=3) as gpool,
            tc.tile_pool(name="o", bufs=3) as opool,
            tc.tile_pool(name="mm", bufs=4, space="PSUM") as psum_mm,
            tc.tile_pool(name="tp", bufs=4, space="PSUM") as psum_tp,
        ):
            a_sb = cpool.tile([KROWS, NSH], f16)
            nc.sync.dma_start(a_sb[:], a_d[:])
            k_sb = cpool.tile([KROWS, S], f16)
            nc.sync.dma_start(k_sb[:], k_d[:])
            qq_sb = cpool.tile([PT, NT], f32)
            nc.sync.dma_start(qq_sb[:], qq_d[:])
            ident = cpool.tile([PT, PT], f32)
            make_identity(nc, ident[:])

            for i in range(NT):
                # 1. distances: m = 2 q.k - |k|^2 for this tile's 128 queries
                m_sb = mpool.tile([PT, S], f32, tag="m")
                lhsT = a_sb[:, i * PT:(i + 1) * PT]
                for j in range(S // 512):
                    ps = psum_mm.tile([PT, 512], f32, tag="mm")
                    nc.tensor.matmul(
                        ps[:], lhsT, k_sb[:, j * 512:(j + 1) * 512],
                        start=True, stop=True,
                    )
                    # 2. PSUM -> SBUF on the scalar engine
                    nc.scalar.copy(m_sb[:, j * 512:(j + 1) * 512], ps[:])

                # 3. top-8 values + indices (we use the first 3)
                top8 = spool.tile([PT, 8], f32, tag="top8")
                nc.vector.max(out=top8[:], in_=m_sb[:])
                idx8 = spool.tile([PT, 8], u32, tag="idx8")
                nc.vector.max_index(out=idx8[:], in_max=top8[:], in_values=m_sb[:])

                # 4. weights: d_c = max((qq + eps) - m_c, tiny) ; w = (1/d)/sum(1/d)
                d3 = spool.tile([PT, 3], f32, tag="d3")
                nc.vector.tensor_scalar(
                    out=d3[:], in0=top8[:, 0:3],
                    scalar1=-1.0, scalar2=qq_sb[:, i:i + 1],
                    op0=mybir.AluOpType.mult, op1=mybir.AluOpType.add,
                )
                # clamp: squared distance is >= 0 mathematically; fp noise can
                # push it slightly negative, which would blow up 1/d
                nc.vector.tensor_scalar(
                    out=d3[:], in0=d3[:], scalar1=1e-9, scalar2=None,
                    op0=mybir.AluOpType.max,
                )
                r3 = spool.tile([PT, 3], f32, tag="r3")
                nc.vector.reciprocal(r3[:], d3[:])
                z = spool.tile([PT, 1], f32, tag="z")
                nc.vector.tensor_reduce(
                    out=z[:], in_=r3[:], axis=mybir.AxisListType.X,
                    op=mybir.AluOpType.add,
                )
                rz = spool.tile([PT, 1], f32, tag="rz")
                nc.vector.reciprocal(rz[:], z[:])
                w3 = spool.tile([PT, 3], f32, tag="w3")
                nc.vector.tensor_scalar(
                    out=w3[:], in0=r3[:], scalar1=rz[:], scalar2=None,
                    op0=mybir.AluOpType.mult,
                )

                # 5. gather v rows for the 3 neighbors and accumulate
                gs = []
                for c in range(3):
                    g = gpool.tile([PT, C], f32, tag=f"g{c}")
                    nc.gpsimd.indirect_dma_start(
                        out=g[:], out_offset=None,
                        in_=v_d[:],
                        in_offset=bass.IndirectOffsetOnAxis(
                            ap=idx8[:, c:c + 1], axis=0,
                        ),
                    )
                    gs.append(g)

                acc = opool.tile([PT, C], f32, tag="acc")
                nc.vector.tensor_scalar(
                    out=acc[:], in0=gs[0][:], scalar1=w3[:, 0:1], scalar2=None,
                    op0=mybir.AluOpType.mult,
                )
                for c in (1, 2):
                    t = opool.tile([PT, C], f32, tag=f"t{c}")
                    nc.scalar.activation(
                        out=t[:], in_=gs[c][:],
                        func=mybir.ActivationFunctionType.Copy,
                        scale=w3[:, c:c + 1],
                    )
                    nc.vector.tensor_tensor(
                        out=acc[:], in0=acc[:], in1=t[:],
                        op=mybir.AluOpType.add,
                    )

                # 6. transpose [q, c] -> [c, q] and store
                for h in range(2):
                    tp = psum_tp.tile([PT, PT], f32, tag="tp")
                    nc.tensor.transpose(
                        out=tp[:], in_=acc[:, h * PT:(h + 1) * PT],
                        identity=ident[:],
                    )
                    ot = opool.tile([PT, PT], f32, tag=f"ot{h}")
                    nc.vector.tensor_copy(out=ot[:], in_=tp[:])
                    nc.sync.dma_start(
                        out_d[h * PT:(h + 1) * PT, i * PT:(i + 1) * PT], ot[:],
                    )

    nc.compile()
    return nc


def _split2(x):
    hi = x.astype(np.float16)
    lo = (x - hi.astype(np.float32)).astype(np.float16)
    return hi, lo


def _split3(x):
    hi = x.astype(np.float16)
    mid = (x - hi.astype(np.float32)).astype(np.float16)
    lo = (x - hi.astype(np.float32) - mid.astype(np.float32)).astype(np.float16)
    return hi, mid, lo


def _host_prep(q, k, v):
    """Build per-core input maps (fp16 3-way-split augmented rows).

    m = 2 q.k - |k|^2 with error ~1e-6 (fp32-class): products kept down to
    2^-33 relative: a_hi*(b_hi,b_mid,b_lo), a_mid*(b_hi,b_mid), a_lo*b_hi,
    plus a 3-way split of -|k|^2 against ones. 6*3 + 3 = 21 rows.
    """
    in_maps = []
    for core in range(NCORES):
        b, h = divmod(core, 2)
        qc = np.ascontiguousarray(q[b, h * NSH:(h + 1) * NSH]).astype(np.float32)
        ah, am, al = _split3(qc)
        ones = np.ones((1, NSH), np.float16)

        kb = (2.0 * k[b]).astype(np.float32)
        bh, bm, bl = _split3(kb)
        kk = -np.sum(k[b].astype(np.float32) * k[b].astype(np.float32), axis=-1)
        ch, cm, cl = _split3(kk)

        pairs = [(ah, bh), (ah, bm), (ah, bl), (am, bh), (am, bm), (al, bh)]
        a = np.concatenate(
            [p[0].T for p in pairs] + [ones, ones, ones], axis=0
        )  # [21, NSH]
        kaug = np.concatenate(
            [p[1].T for p in pairs] + [ch[None], cm[None], cl[None]], axis=0
        )  # [21, S]

        qq = np.sum(qc * qc, axis=-1) + 1e-8  # [NSH]
        qq_t = np.ascontiguousarray(qq.reshape(NT, PT).T)  # [128, NT]

        in_maps.append({
            "a": np.ascontiguousarray(a),
            "kaug": np.ascontiguousarray(kaug),
            "qq": qq_t.astype(np.float32),
            "v": np.ascontiguousarray(v[b]).astype(np.float32),
        })
    return in_maps


LAST_RESULTS = None


def kernel(q, k, v):
    global LAST_RESULTS
    from concourse import bass_utils

    if "nc" not in _CACHE:
        _CACHE["nc"] = _build_bass()
    nc = _CACHE["nc"]

    in_maps = _host_prep(np.asarray(q), np.asarray(k), np.asarray(v))
    res = bass_utils.run_bass_kernel_spmd(
        nc, in_maps, core_ids=list(range(NCORES)),
    )
    LAST_RESULTS = res

    full = np.empty((B, C, N), np.float32)
    for core in range(NCORES):
        b, h = divmod(core, 2)
        full[b, :, h * NSH:(h + 1) * NSH] = res.results[core]["out"]
    return full


# revision 8
# speedup vs baseline: 1.0752x; 1.0752x over previous
"""PointUpsampleAttn (3-NN gather attention) Trainium2 kernel.

Full-input contract: kernel(q, k, v) -> [B, C, N] float32.
  q [4, 16384, 3], k [4, 4096, 3], v [4, 4096, 256]

Sharding: B*N = 65536 queries split across 8 cores (8192 each); core c
handles batch c//2, query half c%2. k/v replicated per-batch (each core
only needs its own batch's k/v). No cross-core reduction.

Per-core kernel, per 128-query tile:
  1. PE matmul (K=11, fp16 hi/lo split of q / 2k / -|k|^2) -> m = 2qk-kk
     in PSUM [128, 4096], fp32-class accuracy at full bf16 PE rate.
  2. ACT copies PSUM -> SBUF.
  3. DVE max8 + max_index -> top-3 m values + s-indices.
  4. weights w = normalize(1/(qq+eps-m_top3)).
  5. 3x indirect DMA gather of v rows; weighted sum; PE transpose to
     [C, n] layout; DMA to output.
"""

import numpy as np

B, N, S, C = 4, 16384, 4096, 256
NCORES = 8
NSH = (B * N) // NCORES   # 8192 queries per core
PT = 128                  # queries per tile (partition dim)
NT = NSH // PT            # 64 tiles
KROWS = 21                # contraction rows of the split matmul

_CACHE = {}


def _build_bass():
    import concourse.bacc as bacc
    import concourse.mybir as mybir
    import concourse.tile as tile
    from concourse import bass
    from concourse.masks import make_identity

    f32 = mybir.dt.float32
    f16 = mybir.dt.float16
    u32 = mybir.dt.uint32

    nc = bacc.Bacc("TRN2", target_bir_lowering=False, debug=False)

    a_d = nc.dram_tensor("a", [KROWS, NSH], f16, kind="ExternalInput").ap()
    k_d = nc.dram_tensor("kaug", [KROWS, S], f16, kind="ExternalInput").ap()
    qq_d = nc.dram_tensor("qq", [PT, NT], f32, kind="ExternalInput").ap()
    v_d = nc.dram_tensor("v", [S, C], f32, kind="ExternalInput").ap()
    out_d = nc.dram_tensor("out", [C, NSH], f32, kind="ExternalOutput").ap()

    with tile.TileContext(nc) as tc:
        with (
            tc.tile_pool(name="const", bufs=1) as cpool,
            tc.tile_pool(name="m", bufs=2) as mpool,
            tc.tile_pool(name="sel", bufs=3) as spool,
            tc.tile_pool(name="g", bufs=3) as gpool,
            tc.tile_pool(name="o", bufs=3) as opool,
            tc.tile_pool(name="mm", bufs=3, space="PSUM") as psum_mm,
            tc.tile_pool(name="tp", bufs=2, space="PSUM") as psum_tp,
        ):
            a_sb = cpool.tile([KROWS, NSH], f16)
            nc.sync.dma_start(a_sb[:], a_d[:])
            k_sb = cpool.tile([KROWS, S], f16)
            nc.sync.dma_start(k_sb[:], k_d[:])
            qq_sb = cpool.tile([PT, NT], f32)
            nc.sync.dma_start(qq_sb[:], qq_d[:])
            ident = cpool.tile([PT, PT], f32)
            make_identity(nc, ident[:])

            for i in range(NT):
                # 1. distances: m = 2 q.k - |k|^2 for this tile's 128 queries
                m_sb = mpool.tile([PT, S], f32, tag="m")
                lhsT = a_sb[:, i * PT:(i + 1) * PT]
                for j in range(S // 1024):
                    ps = psum_mm.tile([PT, 1024], f32, tag="mm")
                    for jj in range(2):
                        nc.tensor.matmul(
                            ps[:, jj * 512:(jj + 1) * 512], lhsT,
                            k_sb[:, j * 1024 + jj * 512:j * 1024 + (jj + 1) * 512],
                            start=True, stop=True,
                        )
                    # 2. PSUM -> SBUF on the scalar engine
                    nc.scalar.copy(m_sb[:, j * 1024:(j + 1) * 1024], ps[:])

                # 3. top-8 values + indices (we use the first 3)
                top8 = spool.tile([PT, 8], f32, tag="top8")
                nc.vector.max(out=top8[:], in_=m_sb[:])
                idx8 = spool.tile([PT, 8], u32, tag="idx8")
                nc.vector.max_index(out=idx8[:], in_max=top8[:], in_values=m_sb[:])

                # 4. weights: d_c = max((qq + eps) - m_c, tiny) ; w = (1/d)/sum(1/d)
                d3 = spool.tile([PT, 3], f32, tag="d3")
                nc.vector.tensor_scalar(
                    out=d3[:], in0=top8[:, 0:3],
                    scalar1=-1.0, scalar2=qq_sb[:, i:i + 1],
                    op0=mybir.AluOpType.mult, op1=mybir.AluOpType.add,
                )
                # clamp: squared distance is >= 0 mathematically; fp noise can
                # push it slightly negative, which would blow up 1/d
                nc.vector.tensor_scalar(
                    out=d3[:], in0=d3[:], scalar1=1e-9, scalar2=None,
                    op0=mybir.AluOpType.max,
                )
                r3 = spool.tile([PT, 3], f32, tag="r3")
                nc.vector.reciprocal(r3[:], d3[:])
                z = spool.tile([PT, 1], f32, tag="z")
                nc.vector.tensor_reduce(
                    out=z[:], in_=r3[:], axis=mybir.AxisListType.X,
                    op=mybir.AluOpType.add,
                )
                rz = spool.tile([PT, 1], f32, tag="rz")
                nc.vector.reciprocal(rz[:], z[:])
                w3 = spool.tile([PT, 3], f32, tag="w3")
                nc.vector.tensor_scalar(
                    out=w3[:], in0=r3[:], scalar1=rz[:], scalar2=None,
                    op0=mybir.AluOpType.mult,
                )

                # 5. gather v rows for the 3 neighbors and accumulate
                gs = []
                for c in range(3):
                    g = gpool.tile([PT, C], f32, tag=f"g{c}")
                    nc.gpsimd.indirect_dma_start(
                        out=g[:], out_offset=None,
                        in_=v_d[:],
                        in_offset=bass.IndirectOffsetOnAxis(
                            ap=idx8[:, c:c + 1], axis=0,
                        ),
                    )
                    gs.append(g)

                # weighted sum on ACT (mults) + GPSIMD (adds): keep DVE free
                acc = opool.tile([PT, C], f32, tag="acc")
                nc.scalar.activation(
                    out=acc[:], in_=gs[0][:],
                    func=mybir.ActivationFunctionType.Copy,
                    scale=w3[:, 0:1],
                )
                for c in (1, 2):
                    t = opool.tile([PT, C], f32, tag=f"t{c}")
                    nc.scalar.activation(
                        out=t[:], in_=gs[c][:],
                        func=mybir.ActivationFunctionType.Copy,
                        scale=w3[:, c:c + 1],
                    )
                    nc.gpsimd.tensor_tensor(
                        out=acc[:], in0=acc[:], in1=t[:],
                        op=mybir.AluOpType.add,
                    )

                # 6. transpose [q, c] -> [c, q] and store
                for h in range(2):
                    tp = psum_tp.tile([PT, PT], f32, tag="tp")
                    nc.tensor.transpose(
                        out=tp[:], in_=acc[:, h * PT:(h + 1) * PT],
                        identity=ident[:],
                    )
                    ot = opool.tile([PT, PT], f32, tag=f"ot{h}")
                    nc.scalar.copy(out=ot[:], in_=tp[:])
                    nc.sync.dma_start(
                        out_d[h * PT:(h + 1) * PT, i * PT:(i + 1) * PT], ot[:],
                    )

    nc.compile()
    return nc


def _split2(x):
    hi = x.astype(np.float16)
    lo = (x - hi.astype(np.float32)).astype(np.float16)
    return hi, lo


def _split3(x):
    hi = x.astype(np.float16)
    mid = (x - hi.astype(np.float32)).astype(np.float16)
    lo = (x - hi.astype(np.float32) - mid.astype(np.float32)).astype(np.float16)
    return hi, mid, lo


def _host_prep(q, k, v):
    """Build per-core input maps (fp16 3-way-split augmented rows).

    m = 2 q.k - |k|^2 with error ~1e-6 (fp32-class): products kept down to
    2^-33 relative: a_hi*(b_hi,b_mid,b_lo), a_mid*(b_hi,b_mid), a_lo*b_hi,
    plus a 3-way split of -|k|^2 against ones. 6*3 + 3 = 21 rows.
    """
    in_maps = []
    for core in range(NCORES):
        b, h = divmod(core, 2)
        qc = np.ascontiguousarray(q[b, h * NSH:(h + 1) * NSH]).astype(np.float32)
        ah, am, al = _split3(qc)
        ones = np.ones((1, NSH), np.float16)

        kb = (2.0 * k[b]).astype(np.float32)
        bh, bm, bl = _split3(kb)
        kk = -np.sum(k[b].astype(np.float32) * k[b].astype(np.float32), axis=-1)
        ch, cm, cl = _split3(kk)

        pairs = [(ah, bh), (ah, bm), (ah, bl), (am, bh), (am, bm), (al, bh)]
        a = np.concatenate(
            [p[0].T for p in pairs] + [ones, ones, ones], axis=0
        )  # [21, NSH]
        kaug = np.concatenate(
            [p[1].T for p in pairs] + [ch[None], cm[None], cl[None]], axis=0
        )  # [21, S]

        qq = np.sum(qc * qc, axis=-1) + 1e-8  # [NSH]
        qq_t = np.ascontiguousarray(qq.reshape(NT, PT).T)  # [128, NT]

        in_maps.append({
            "a": np.ascontiguousarray(a),
            "kaug": np.ascontiguousarray(kaug),
            "qq": qq_t.astype(np.float32),
            "v": np.ascontiguousarray(v[b]).astype(np.float32),
        })
    return in_maps


LAST_RESULTS = None


def kernel(q, k, v):
    global LAST_RESULTS
    from concourse import bass_utils

    if "nc" not in _CACHE:
        _CACHE["nc"] = _build_bass()
    nc = _CACHE["nc"]

    in_maps = _host_prep(np.asarray(q), np.asarray(k), np.asarray(v))
    res = bass_utils.run_bass_kernel_spmd(
        nc, in_maps, core_ids=list(range(NCORES)),
    )
    LAST_RESULTS = res

    full = np.empty((B, C, N), np.float32)
    for core in range(NCORES):
        b, h = divmod(core, 2)
        full[b, :, h * NSH:(h + 1) * NSH] = res.results[core]["out"]
    return full


# revision 15
# speedup vs baseline: 1.1618x; 1.0806x over previous
"""PointUpsampleAttn (3-NN gather attention) Trainium2 kernel.

Full-input contract: kernel(q, k, v) -> [B, C, N] float32.
  q [4, 16384, 3], k [4, 4096, 3], v [4, 4096, 256]

Sharding: B*N = 65536 queries split across 8 cores (8192 each); core c
handles batch c//2, query half c%2. k/v replicated per-batch (each core
only needs its own batch's k/v). No cross-core reduction.

Per-core kernel, per 128-query tile:
  1. PE matmul (K=11, fp16 hi/lo split of q / 2k / -|k|^2) -> m = 2qk-kk
     in PSUM [128, 4096], fp32-class accuracy at full bf16 PE rate.
  2. ACT copies PSUM -> SBUF.
  3. DVE max8 + max_index -> top-3 m values + s-indices.
  4. weights w = normalize(1/(qq+eps-m_top3)).
  5. 3x indirect DMA gather of v rows; weighted sum; PE transpose to
     [C, n] layout; DMA to output.
"""

import numpy as np

B, N, S, C = 4, 16384, 4096, 256
NCORES = 8
NSH = (B * N) // NCORES   # 8192 queries per core
PT = 128                  # queries per tile (partition dim)
NT = NSH // PT            # 64 tiles
KROWS = 21                # contraction rows of the split matmul

_CACHE = {}


def _build_bass():
    import concourse.bacc as bacc
    import concourse.mybir as mybir
    import concourse.tile as tile
    from concourse import bass
    from concourse.masks import make_identity

    f32 = mybir.dt.float32
    f16 = mybir.dt.float16
    u32 = mybir.dt.uint32

    nc = bacc.Bacc("TRN2", target_bir_lowering=False, debug=False)

    a_d = nc.dram_tensor("a", [KROWS, NSH], f16, kind="ExternalInput").ap()
    k_d = nc.dram_tensor("kaug", [KROWS, S], f16, kind="ExternalInput").ap()
    qq_d = nc.dram_tensor("qq", [PT, NT], f32, kind="ExternalInput").ap()
    v_d = nc.dram_tensor("v", [S, C], f32, kind="ExternalInput").ap()
    out_d = nc.dram_tensor("out", [C, NSH], f32, kind="ExternalOutput").ap()

    with tile.TileContext(nc) as tc:
        with (
            tc.tile_pool(name="const", bufs=1) as cpool,
            tc.tile_pool(name="m", bufs=2) as mpool,
            tc.tile_pool(name="sel", bufs=3) as spool,
            tc.tile_pool(name="g", bufs=3) as gpool,
            tc.tile_pool(name="o", bufs=3) as opool,
            tc.tile_pool(name="mm", bufs=3, space="PSUM") as psum_mm,
            tc.tile_pool(name="tp", bufs=2, space="PSUM") as psum_tp,
        ):
            a_sb = cpool.tile([KROWS, NSH], f16)
            nc.sync.dma_start(a_sb[:], a_d[:])
            k_sb = cpool.tile([KROWS, S], f16)
            nc.sync.dma_start(k_sb[:], k_d[:])
            qq_sb = cpool.tile([PT, NT], f32)
            nc.sync.dma_start(qq_sb[:], qq_d[:])
            ident = cpool.tile([PT, PT], f32)
            make_identity(nc, ident[:])
            eps1 = cpool.tile([PT, 1], f32)
            nc.gpsimd.memset(eps1[:], 1e-9)

            for i in range(NT):
                # 1. distances: m = 2 q.k - |k|^2 for this tile's 128 queries
                m_sb = mpool.tile([PT, S], f32, tag="m")
                lhsT = a_sb[:, i * PT:(i + 1) * PT]
                for j in range(S // 1024):
                    ps = psum_mm.tile([PT, 1024], f32, tag="mm")
                    for jj in range(2):
                        nc.tensor.matmul(
                            ps[:, jj * 512:(jj + 1) * 512], lhsT,
                            k_sb[:, j * 1024 + jj * 512:j * 1024 + (jj + 1) * 512],
                            start=True, stop=True,
                        )
                    # 2. PSUM -> SBUF on the scalar engine
                    nc.scalar.copy(m_sb[:, j * 1024:(j + 1) * 1024], ps[:])

                # 3. top-8 values + indices (we use the first 3)
                top8 = spool.tile([PT, 8], f32, tag="top8")
                nc.vector.max(out=top8[:], in_=m_sb[:])
                idx8 = spool.tile([PT, 8], u32, tag="idx8")
                nc.vector.max_index(out=idx8[:], in_max=top8[:], in_values=m_sb[:])

                # 4. weights: d_c = max((qq + eps) - m_c, tiny) ; w = (1/d)/sum(1/d)
                # elementwise steps on ACT (relu gives the clamp); recip on DVE.
                # d3 = relu(-m + (qq + eps - tiny)) + tiny
                d3r = spool.tile([PT, 3], f32, tag="d3r")
                nc.scalar.activation(
                    out=d3r[:], in_=top8[:, 0:3],
                    func=mybir.ActivationFunctionType.Relu,
                    scale=-1.0, bias=qq_sb[:, i:i + 1],
                )
                # d3r >= 0, so relu(d3r + eps) == d3r + eps (the floor)
                d3 = spool.tile([PT, 3], f32, tag="d3")
                nc.scalar.activation(
                    out=d3[:], in_=d3r[:],
                    func=mybir.ActivationFunctionType.Relu,
                    bias=eps1[:],
                )
                r3 = spool.tile([PT, 3], f32, tag="r3")
                nc.vector.reciprocal(r3[:], d3[:])
                z = spool.tile([PT, 1], f32, tag="z")
                nc.vector.tensor_reduce(
                    out=z[:], in_=r3[:], axis=mybir.AxisListType.X,
                    op=mybir.AluOpType.add,
                )
                rz = spool.tile([PT, 1], f32, tag="rz")
                nc.vector.reciprocal(rz[:], z[:])
                w3 = spool.tile([PT, 3], f32, tag="w3")
                nc.scalar.activation(
                    out=w3[:], in_=r3[:],
                    func=mybir.ActivationFunctionType.Copy,
                    scale=rz[:],
                )

                # 5. gather v rows (one indirect DMA per neighbor; multi-wide
                # offset APs mis-execute on hardware), then weighted sum
                gs = []
                for c in range(3):
                    g = gpool.tile([PT, C], f32, tag=f"g{c}")
                    nc.gpsimd.indirect_dma_start(
                        out=g[:], out_offset=None,
                        in_=v_d[:],
                        in_offset=bass.IndirectOffsetOnAxis(
                            ap=idx8[:, c:c + 1], axis=0,
                        ),
                    )
                    gs.append(g)

                acc = opool.tile([PT, C], f32, tag="acc")
                nc.scalar.activation(
                    out=acc[:], in_=gs[0][:],
                    func=mybir.ActivationFunctionType.Copy,
                    scale=w3[:, 0:1],
                )
                for c in (1, 2):
                    t = opool.tile([PT, C], f32, tag=f"t{c}")
                    nc.scalar.activation(
                        out=t[:], in_=gs[c][:],
                        func=mybir.ActivationFunctionType.Copy,
                        scale=w3[:, c:c + 1],
                    )
                    nc.gpsimd.tensor_tensor(
                        out=acc[:], in0=acc[:], in1=t[:],
                        op=mybir.AluOpType.add,
                    )

                # 6. transpose [q, c] -> [c, q] and store
                for h in range(2):
                    tp = psum_tp.tile([PT, PT], f32, tag="tp")
                    nc.tensor.transpose(
                        out=tp[:], in_=acc[:, h * PT:(h + 1) * PT],
                        identity=ident[:],
                    )
                    ot = opool.tile([PT, PT], f32, tag=f"ot{h}")
                    nc.scalar.copy(out=ot[:], in_=tp[:])
                    nc.sync.dma_start(
                        out_d[h * PT:(h + 1) * PT, i * PT:(i + 1) * PT], ot[:],
                    )

    nc.compile()
    return nc


def _split2(x):
    hi = x.astype(np.float16)
    lo = (x - hi.astype(np.float32)).astype(np.float16)
    return hi, lo


def _split3(x):
    hi = x.astype(np.float16)
    mid = (x - hi.astype(np.float32)).astype(np.float16)
    lo = (x - hi.astype(np.float32) - mid.astype(np.float32)).astype(np.float16)
    return hi, mid, lo


def _host_prep(q, k, v):
    """Build per-core input maps (fp16 3-way-split augmented rows).

    m = 2 q.k - |k|^2 with error ~1e-6 (fp32-class): products kept down to
    2^-33 relative: a_hi*(b_hi,b_mid,b_lo), a_mid*(b_hi,b_mid), a_lo*b_hi,
    plus a 3-way split of -|k|^2 against ones. 6*3 + 3 = 21 rows.
    """
    in_maps = []
    for core in range(NCORES):
        b, h = divmod(core, 2)
        qc = np.ascontiguousarray(q[b, h * NSH:(h + 1) * NSH]).astype(np.float32)
        ah, am, al = _split3(qc)
        ones = np.ones((1, NSH), np.float16)

        kb = (2.0 * k[b]).astype(np.float32)
        bh, bm, bl = _split3(kb)
        kk = -np.sum(k[b].astype(np.float32) * k[b].astype(np.float32), axis=-1)
        ch, cm, cl = _split3(kk)

        pairs = [(ah, bh), (ah, bm), (ah, bl), (am, bh), (am, bm), (al, bh)]
        a = np.concatenate(
            [p[0].T for p in pairs] + [ones, ones, ones], axis=0
        )  # [21, NSH]
        kaug = np.concatenate(
            [p[1].T for p in pairs] + [ch[None], cm[None], cl[None]], axis=0
        )  # [21, S]

        qq = np.sum(qc * qc, axis=-1) + 1e-8  # [NSH]
        qq_t = np.ascontiguousarray(qq.reshape(NT, PT).T)  # [128, NT]

        in_maps.append({
            "a": np.ascontiguousarray(a),
            "kaug": np.ascontiguousarray(kaug),
            "qq": qq_t.astype(np.float32),
            "v": np.ascontiguousarray(v[b]).astype(np.float32),
        })
    return in_maps


LAST_RESULTS = None


def _ensure_ntff_hook_importable():
    """bass_utils imports antenv.axon_hooks when tracing is requested; some
    images lack that module. Provide it (wired to libaxon_pjrt if present)."""
    import sys, types
    try:
        import antenv.axon_hooks  # noqa: F401
        return
    except Exception:
        pass
    try:
        import antenv
    except Exception:
        return
    mod = types.ModuleType("antenv.axon_hooks")
    try:
        from trn_agent_boot.trn_boot import _ntff_profile_via_ctypes
        _hook = _ntff_profile_via_ctypes("/opt/axon/libaxon_pjrt.so")
    except Exception:
        _hook = None
    mod.get_axon_ntff_profile_hook = lambda: _hook
    mod.set_axon_ntff_profile_hook = lambda h: None
    sys.modules["antenv.axon_hooks"] = mod
    antenv.axon_hooks = mod


def kernel(q, k, v):
    global LAST_RESULTS
    _ensure_ntff_hook_importable()
    from concourse import bass_utils

    if "nc" not in _CACHE:
        _CACHE["nc"] = _build_bass()
    nc = _CACHE["nc"]

    in_maps = _host_prep(np.asarray(q), np.asarray(k), np.asarray(v))
    res = bass_utils.run_bass_kernel_spmd(
        nc, in_maps, core_ids=list(range(NCORES)),
    )
    LAST_RESULTS = res

    full = np.empty((B, C, N), np.float32)
    for core in range(NCORES):
        b, h = divmod(core, 2)
        full[b, :, h * NSH:(h + 1) * NSH] = res.results[core]["out"]
    return full
